# revision 6
# baseline (speedup 1.0000x reference)
"""Trainium2 Bass kernel for nn_BasicS2ConvV2 (dual-fp8 DoubleRow).

Computes out[b,d,p,r] = sum_{c,k,a} W_eff[d,c,k,a,r] * x[b,c,k,p,a], where
W_eff[d,c,k,a,r] = W[d, c, M_idx[k,a,r]] is a pure index-gather of the small
parameter tensor W (materialized on the host).

Device strategy (per NeuronCore, x sharded over p into 8 slices of 1024):
  - The einsum is a matmul with contraction (c,k,a)=4992 = 39 K-tiles of 128.
    M packs (rsub, d) = 4 r's x 32 d's = 128 output partitions; 3 r-groups
    cover r=12.  The moving free dim is p (PT=512 = one fp32 PSUM bank).
  - Matmuls run in fp8 DoubleRow (dual-fp8) mode: each logical contraction
    row is SPLIT into a pair of e4m3 rows, so one DoubleRow matmul contracts
    a full 128-logical-row tile in ~half the cycles of bf16.
      x rows:  x = xa + xb with xa = e4m3(0.75*x), xb = e4m3(x - xa).
      W rows:  each weight W becomes an e4m3 pair (w0, w1) chosen by lattice
               search to minimize |0.75*(w0-W) + 0.25*(w1-W)|, giving ~4x the
               effective precision of a single e4m3 on BOTH operands.
    Result: w0*xa + w1*xb ~= W*x with ~1.1e-2 scale-relative max error.
  - Weight reuse: each (rg, t) weight tile feeds the 2 p-tile matmuls of a
    batch back-to-back (6 PSUM banks: 3 r-groups x 2 p-tiles), halving the
    LDWEIGHTS rate, which otherwise exceeds the dual-fp8 matmul issue rate.
  - DMA: x p-half i=0 rides the sync (SP) queue, i=1 the vector (DVE) queue,
    W + outputs the scalar (ACT) queue, so no single queue is the bottleneck
    at the dual-fp8 matmul rate.
  - Output is written as out[b, rg, m=(rsub*32+d), p] bf16; the host
    transposes to [b, d, p, r] and concatenates the p-shards.
"""

import numpy as np
import ml_dtypes

# Problem shapes (hardcoded; harness runs kernel.py standalone).
B = 2
DIN = 32
DOUT = 32
KK = 13          # kernel size
A = 12           # anchor size
R = 12           # rotation copies
N_PARAM = 36
P_FULL = 8192
N_CORES = 8
P_LOC = P_FULL // N_CORES       # 1024 points per core
CK = DIN * KK                   # 416 contraction rows per a
PT = 512                        # p tile (= 512 fp32 PSUM bank, max moving)
RG = 3                          # r groups (4 r's each)
RSUB = 4
NT = 39                         # lhsT tiles per r-group: 12a x 3ch + 3 packed
NPT = P_LOC // PT               # 2 p tiles per core
XCH = 8                         # x DMA chunk size in tiles
ALPHA = 0.75                    # x hi-component fraction

F8 = ml_dtypes.float8_e4m3      # TRN FP8_EXP4 (max normal 240)

_NC_CACHE = None


def _build_nc(pt=PT, repeat=1):
    import concourse.bacc as bacc
    import concourse.mybir as mybir
    import concourse.tile as tile

    f8 = mybir.dt.float8e4
    bf16 = mybir.dt.bfloat16
    f32 = mybir.dt.float32
    DR = mybir.MatmulPerfMode.DoubleRow

    nc = bacc.Bacc("TRN2", target_bir_lowering=False, debug=False,
                   num_devices=N_CORES)
    xp_in = nc.dram_tensor("xp", [B, NPT, 128, NT, 2, pt], f8,
                           kind="ExternalInput")
    wef_in = nc.dram_tensor("wef", [128, RG, NT, 2, 128], f8,
                            kind="ExternalInput")
    out_t = nc.dram_tensor("out", [B, RG, 128, P_LOC], bf16,
                           kind="ExternalOutput")

    with tile.TileContext(nc) as tc:
        with (
            tc.tile_pool(name="wpool", bufs=2) as wpool,
            tc.tile_pool(name="xpool", bufs=4) as xpool,
            tc.tile_pool(name="spool", bufs=1) as spool,
            tc.tile_pool(name="opool", bufs=3) as opool,
            tc.tile_pool(name="pspool", bufs=1, space="PSUM") as pspool,
        ):
          # PE warm-up: dummy matmuls on a zeroed scratch tile fill the
          # HAM cold window while the first W/x DMAs land.
          scr = spool.tile([128, 640], f8, tag="scr")
          nc.vector.memset(scr[:], 0)
          ps_d = pspool.tile([128, pt], f32, tag="psd")
          for _ in range(10):
              nc.tensor.matmul(ps_d[:, :], scr[:, :128], scr[:, 128:128 + pt],
                               start=True, stop=True)

          for _rep in range(repeat):
            W_sb = wpool.tile([128, RG, NT, 2, 128], f8, tag="wsb")
            # W on the scalar queue, rg-major chunks so rg0 is ready first.
            for rg in range(RG):
                for (c0, c1) in ((0, 10), (10, 20), (20, 30), (30, NT)):
                    nc.scalar.dma_start(W_sb[:, rg, c0:c1],
                                        wef_in[:, rg, c0:c1])

            for b in range(B):
                # x arrives in t-chunks; the t-outer matmul loop consumes
                # each chunk exactly once, so chunks rotate through a small
                # pool.  p-half 0 rides the sync queue, p-half 1 gpsimd.
                chunks = []
                t0 = 0
                while t0 < NT:
                    t1 = min(t0 + XCH, NT)
                    xc = [xpool.tile([128, t1 - t0, 2, pt], f8, tag=f"x{i}",
                                     name=f"x{i}") for i in range(NPT)]
                    nc.sync.dma_start(xc[0][:], xp_in[b, 0, :, t0:t1])
                    nc.gpsimd.dma_start(xc[1][:], xp_in[b, 1, :, t0:t1])
                    chunks.append((t0, t1, xc))
                    t0 = t1

                ps = [[pspool.tile([128, pt], f32, tag=f"ps{rg}{i}",
                                   name=f"ps{rg}{i}")
                       for i in range(NPT)] for rg in range(RG)]
                for (t0, t1, xc) in chunks:
                    for t in range(t0, t1):
                        for rg in range(RG):
                            for i in range(NPT):
                                nc.tensor.matmul(
                                    ps[rg][i][:, :],
                                    W_sb[:, rg, t],
                                    xc[i][:, t - t0],
                                    start=(t == 0), stop=(t == NT - 1),
                                    perf_mode=DR)
                # drain all r-groups (bf16); overlaps the next b's matmuls
                for rg in range(RG):
                    for i in range(NPT):
                        ot = opool.tile([128, pt], bf16, tag="ot")
                        nc.any.tensor_copy(ot[:], ps[rg][i][:])
                        nc.scalar.dma_start(
                            out_t[b, rg, :, i * pt:(i + 1) * pt], ot[:])

    nc.compile()
    return nc


def _get_nc():
    global _NC_CACHE
    if _NC_CACHE is None:
        _NC_CACHE = _build_nc()
    return _NC_CACHE


def _f8_codes():
    """All finite e4m3 values, sorted (positive and negative)."""
    codes = np.arange(256, dtype=np.uint8).view(F8).astype(np.float32)
    return np.unique(codes[np.isfinite(codes)])


def _encode_w_pair(Wv):
    """Per-element (w0, w1) e4m3 pair minimizing |a*d0 + (1-a)*d1|."""
    uniq, inv = np.unique(Wv, return_inverse=True)
    codes = _f8_codes()
    idx0 = np.searchsorted(codes, uniq)
    RAD = 3
    cand = np.stack([codes[np.clip(idx0 + k, 0, codes.size - 1)]
                     for k in range(-RAD, RAD + 1)], axis=1)   # [U, 2R+1]
    c0 = cand[:, :, None]
    c1 = cand[:, None, :]
    obj = np.abs(ALPHA * (c0 - uniq[:, None, None])
                 + (1 - ALPHA) * (c1 - uniq[:, None, None]))
    flat = obj.reshape(uniq.size, -1).argmin(axis=1)
    nc_ = cand.shape[1]
    w0u = np.take_along_axis(cand, (flat // nc_)[:, None], 1)[:, 0]
    w1u = np.take_along_axis(cand, (flat % nc_)[:, None], 1)[:, 0]
    return (w0u[inv].reshape(Wv.shape).astype(F8),
            w1u[inv].reshape(Wv.shape).astype(F8))


def _host_weights(W, idx_map, idxs_k, idxs_a):
    """Build e4m3 dual lhsT pack wef[q, rg, t, j, m=(rsub*32+d)].

    Tiles t per r-group: t = a*3+ch (ch<3, rows q = ck=ch*128+q) for the
    full ck chunks; t = 36+j for the packed remainder, whose row q = 32g+qq
    holds ck = 384+qq at a = 4j+g.  j indexes the (w0, w1) e4m3 pair.
    """
    W = np.asarray(W, dtype=np.float32)
    idx_map = np.asarray(idx_map).astype(np.int64)
    idxs_k = np.asarray(idxs_k).astype(np.int64)
    idxs_a = np.asarray(idxs_a).astype(np.int64)

    Wr = W[:, :, idx_map].reshape(DOUT, DIN, KK, A)          # [d,c,k,a]
    a2 = idxs_a                                              # [K,A,R]
    k_ix = np.arange(KK)[:, None, None]
    r_ix = np.arange(R)[None, None, :]
    k2 = idxs_k[k_ix, a2, r_ix]                              # [K,A,R]
    W_eff = Wr[:, :, k2, a2]                                 # [d,c,K,A,R]

    # -> [ck, a, rg, m] with ck = c*13 + k, m = rsub*32 + d, r = rg*4+rsub
    Wf = np.ascontiguousarray(W_eff.transpose(1, 2, 3, 4, 0)).reshape(
        CK, A, R, DOUT).reshape(CK, A, RG, RSUB * DOUT)

    wefA = Wf[:384].reshape(3, 128, A, RG, 128)              # [ch,q,a,rg,m]
    wefA = wefA.transpose(1, 3, 2, 0, 4).reshape(128, RG, 36, 128)

    wefB = Wf[384:].reshape(32, 3, 4, RG, 128)               # [qq,j,g,rg,m]
    wefB = wefB.transpose(2, 0, 3, 1, 4).reshape(128, RG, 3, 128)

    wef = np.concatenate([wefA, wefB], axis=2)               # [128,RG,39,128]
    w0, w1 = _encode_w_pair(np.ascontiguousarray(wef))
    return np.ascontiguousarray(
        np.stack([w0, w1], axis=3))                          # [128,RG,39,2,128]


def _pack_x_layout(xr):
    """xr [B, CK, P_LOC, A] (any dtype) -> [B, NPT, 128, NT, PT] tiles."""
    xA = xr[:, :384].reshape(B, 3, 128, NPT, PT, A)          # [b,ch,q,i,p,a]
    xA = xA.transpose(0, 3, 2, 5, 1, 4).reshape(B, NPT, 128, 36, PT)
    xB = xr[:, 384:].reshape(B, 32, NPT, PT, 3, 4)           # [b,qq,i,p,j,g]
    xB = xB.transpose(0, 2, 5, 1, 4, 3).reshape(B, NPT, 128, 3, PT)
    return np.concatenate([xA, xB], axis=3)                  # [B,NPT,128,NT,PT]


def _pack_x(x):
    """x [B,DIN,KK,P,A] fp32 -> per-core xp [B,NPT,128,NT,2,PT] e4m3."""
    xf = np.asarray(x, dtype=np.float32).reshape(B, CK, P_FULL, A)
    xa = (ALPHA * xf).astype(F8)
    xb = (xf - xa.astype(np.float32)).astype(F8)
    packs = []
    for core in range(N_CORES):
        sl = slice(core * P_LOC, (core + 1) * P_LOC)
        pa = _pack_x_layout(xa[:, :, sl, :])
        pb = _pack_x_layout(xb[:, :, sl, :])
        packs.append(np.ascontiguousarray(
            np.stack([pa, pb], axis=4)))                     # [B,NPT,128,NT,2,PT]
    return packs


def _prepare_in_maps(inputs):
    wef = _host_weights(inputs["W"], inputs["idx_map"],
                        inputs["idxs_k"], inputs["idxs_a"])
    packs = _pack_x(inputs["x"])
    return [{"xp": packs[core], "wef": wef} for core in range(N_CORES)]


def _decode_out(core_outs):
    """core_outs: list of per-core 'out' arrays [B,RG,128,P_LOC] -> full."""
    shards = []
    for od in core_outs:
        od = np.asarray(od).astype(np.float32)
        od = od.reshape(B, RG, RSUB, DOUT, P_LOC)
        od = od.transpose(0, 3, 4, 1, 2).reshape(B, DOUT, P_LOC, R)
        shards.append(od)
    return np.ascontiguousarray(np.concatenate(shards, axis=2))


def _run(inputs, trace=False):
    from concourse.bass_utils import run_bass_kernel_spmd

    in_maps = _prepare_in_maps(inputs)
    nc = _get_nc()
    res = run_bass_kernel_spmd(nc, in_maps, core_ids=list(range(N_CORES)),
                               trace=trace)
    out = _decode_out([res.results[c]["out"] for c in range(N_CORES)])
    return out, res


def kernel(**inputs):
    out, _ = _run(inputs, trace=False)
    return out
